# revision 27
# baseline (speedup 1.0000x reference)
"""Trainium2 Bass kernel for MockFP8Linear: out = x @ (W * block_scale)^T.

Strategy: data-parallel over tokens across 8 NeuronCores (no collectives).

Layout: the PE contracts along the partition dim, so both operands need
in_features on partitions, but both x and W are stored in_features-
innermost.
  - weight: fed to the device pre-transposed ([in, out] layout — a host-side
    np.ascontiguousarray(weight.T), layout prep only). The dequant scaling
    + bf16 cast happen on-device in one DVE tensor_tensor multiply per
    half-row tile, using a stride-0 broadcast AP for the per-128x128-block
    scales. W^T (bf16, 8 MB) stays resident in SBUF.
  - x: cast f32->bf16 on DVE, then 128x128 blocks are transposed on the
    TensorEngine (transpose-mode matmul against an identity, ~60 ns each
    when batched back-to-back), evicted from PSUM to SBUF by ACT in
    4-block batches. The transposes for token tile tt+1 are woven into
    tile tt's matmul stream so the PE never idles long enough for the HAM
    clock gate to re-throttle.

Main compute runs as two passes over output halves (pass A: o[0:1024]
with the x pipeline woven in, pass B: o[1024:2048] as a pure matmul
stream over the resident x^T tiles — measured at the N=512 issue-rate
floor). lhsT(=x^T block, stationary) @ rhs(=W^T slice, moving, N=512)
bf16 matmuls accumulate fp32 in PSUM over the 16 k-blocks; DVE/ACT evict
to SBUF, DMA out. PSUM accumulator and transpose tiles share one
8-buffer pool.
"""

import os
import sys

import numpy as np

for _p in ("/opt/trn_rl_repo", "/root/.axon_site/_ro/trn_rl_repo"):
    if os.path.isdir(_p) and _p not in sys.path:
        sys.path.append(_p)

TOKENS, IN_F, OUT_F = 16384, 2048, 2048
NCORES = 8
TSH = TOKENS // NCORES  # tokens per core
P = 128
KB = IN_F // P  # contraction blocks
TB = TSH // P  # token tiles per core
OBL = OUT_F // P  # out_features blocks (scale granularity)
NCH = OUT_F // 512  # psum chunks of the output row-tile

_cached = None


def _build():
    from contextlib import ExitStack

    import concourse.tile as tile
    from concourse import bacc, mybir
    from concourse.bass import ds
    from concourse.masks import make_identity

    f32 = mybir.dt.float32
    bf16 = mybir.dt.bfloat16

    nc = bacc.Bacc("TRN2", target_bir_lowering=False, debug=False, num_devices=NCORES)
    x_d = nc.dram_tensor("x", [TSH, IN_F], f32, kind="ExternalInput").ap()
    wt_d = nc.dram_tensor("wt", [IN_F, OUT_F], bf16, kind="ExternalInput").ap()
    s_d = nc.dram_tensor("s", [P, KB, OBL], f32, kind="ExternalInput").ap()
    o_d = nc.dram_tensor("out", [TSH, OUT_F], f32, kind="ExternalOutput").ap()

    with tile.TileContext(nc) as tc:
        with ExitStack() as ctx:
            const = ctx.enter_context(tc.tile_pool(name="const", bufs=1))
            scales = const.tile([P, KB, OBL], f32)
            nc.scalar.dma_start(scales[:], s_d[:])
            ident = const.tile([P, P], bf16)
            make_identity(nc, ident)

            wT_pool = ctx.enter_context(tc.tile_pool(name="wT", bufs=1))
            wTs = [wT_pool.tile([P, OUT_F], bf16, name=f"wT_{ib}") for ib in range(KB)]

            wnat_pool = ctx.enter_context(tc.tile_pool(name="wnat", bufs=3))
            xnat_pool = ctx.enter_context(tc.tile_pool(name="xnat", bufs=3))
            xbf_pool = ctx.enter_context(tc.tile_pool(name="xbf", bufs=3))
            xT_pool = ctx.enter_context(tc.tile_pool(name="xT", bufs=1))
            outsb_pool = ctx.enter_context(tc.tile_pool(name="outsb", bufs=2))
            ops_pool = ctx.enter_context(tc.tile_pool(name="ops", bufs=8, space="PSUM"))
            tps_pool = ops_pool

            def emit_w_half(ib, h):
                wnat = wnat_pool.tile(
                    [P, OUT_F // 2], bf16, tag="wnat", name=f"wnat_{ib}_{h}"
                )
                nc.scalar.dma_start(
                    wnat[:], wt_d[ds(ib * P, P), ds(h * (OUT_F // 2), OUT_F // 2)]
                )
                nc.vector.tensor_tensor(
                    out=wTs[ib][:, ds(h * (OUT_F // 2), OUT_F // 2)].rearrange(
                        "p (b c) -> p b c", c=P
                    ),
                    in0=wnat[:].rearrange("p (b c) -> p b c", c=P),
                    in1=scales[:, ib, ds(h * (OBL // 2), OBL // 2), None].broadcast_to(
                        [P, OBL // 2, P]
                    ),
                    op=mybir.AluOpType.mult,
                )

            # ---- two passes over output halves: pass A computes o[0:1024]
            # for all token tiles with tt+1's load/cast/PE-transposes woven
            # into tt's matmul stream (x^T tiles stay resident); pass B is a
            # pure matmul stream over o[1024:2048]. First MMs need only the
            # first 256 KB of W; W's h=1 halves stream in during pass A. ----
            xbfs = {}
            xTs = [xT_pool.tile([P, IN_F], bf16, name=f"xT_{t}") for t in range(TB)]

            def emit_load_cast(t, chunks=1):
                c = IN_F // chunks
                xnat = xnat_pool.tile([P, IN_F], f32, tag="xnat", name=f"xnat_{t}")
                xbf = xbf_pool.tile([P, IN_F], bf16, tag="xbf", name=f"xbf_{t}")
                for j in range(chunks):
                    nc.sync.dma_start(
                        xnat[:, ds(j * c, c)], x_d[ds(t * P, P), ds(j * c, c)]
                    )
                    nc.vector.tensor_copy(xbf[:, ds(j * c, c)], xnat[:, ds(j * c, c)])
                xbfs[t] = xbf

            def emit_transposes(t, q):
                # quarter q: transpose blocks 4q..4q+3 of token tile t
                tps = tps_pool.tile([P, 4 * P], bf16, tag="ops", name=f"tps_{t}_{q}")
                for j in range(4):
                    ib = 4 * q + j
                    nc.tensor.transpose(
                        tps[:, ds(j * P, P)],
                        xbfs[t][:, ds(ib * P, P)],
                        ident[:],
                    )
                nc.scalar.copy(xTs[t][:, ds(q * 4 * P, 4 * P)], tps[:])

            # prologue: token tile 0 in 512-col chunks so the first
            # transposes start as early as possible
            emit_load_cast(0, chunks=4)
            for q in range(4):
                emit_transposes(0, q)
            emit_load_cast(1)
            for h in range(2):
                for ib in range(KB):
                    emit_w_half(ib, h)

            def half_pass(h, weave):
                for tt in range(TB):
                    xT = xTs[tt]
                    psum = [
                        ops_pool.tile(
                            [P, 512], f32, tag="ops", name=f"ops_{h}_{tt}_{nb}"
                        )
                        for nb in range(2)
                    ]
                    for ib in range(KB):
                        lhsT = xT[:, ds(ib * P, P)]
                        for nb in range(2):
                            nc.tensor.matmul(
                                psum[nb][:],
                                lhsT=lhsT,
                                rhs=wTs[ib][:, ds(h * 1024 + nb * 512, 512)],
                                start=(ib == 0),
                                stop=(ib == KB - 1),
                            )
                        if weave and tt + 1 < TB and ib % 4 == 3:
                            emit_transposes(tt + 1, ib // 4)

                    if weave and tt + 2 < TB:
                        emit_load_cast(tt + 2)
                    outsb = outsb_pool.tile(
                        [P, 1024], f32, tag="outsb", name=f"osb_{h}_{tt}"
                    )
                    # split the eviction across DVE and ACT so the last
                    # tile's drain is half as long
                    nc.vector.tensor_copy(outsb[:, ds(0, 512)], psum[0][:])
                    nc.scalar.copy(outsb[:, ds(512, 512)], psum[1][:])
                    nc.sync.dma_start(
                        o_d[ds(tt * P, P), ds(h * 1024, 1024)], outsb[:]
                    )

            half_pass(0, weave=True)
            half_pass(1, weave=False)

    nc.compile()
    return nc


def _get_compiled():
    global _cached
    if _cached is None:
        _cached = _build()
    return _cached


def _ensure_ntff_hook():
    """Register the axon NTFF profile hook (boot skips it when
    antenv.axon_hooks is absent from the image). Only needed for trace=True."""
    import sys as _sys
    import types as _types

    if "antenv.axon_hooks" not in _sys.modules:
        import antenv

        mod = _types.ModuleType("antenv.axon_hooks")
        mod._hook = None

        def set_axon_ntff_profile_hook(h):
            mod._hook = h

        def get_axon_ntff_profile_hook():
            return mod._hook

        mod.set_axon_ntff_profile_hook = set_axon_ntff_profile_hook
        mod.get_axon_ntff_profile_hook = get_axon_ntff_profile_hook
        _sys.modules["antenv.axon_hooks"] = mod
        antenv.axon_hooks = mod
    mod = _sys.modules["antenv.axon_hooks"]
    if mod._hook is None:
        from trn_agent_boot.trn_boot import _ntff_profile_via_ctypes

        hook = _ntff_profile_via_ctypes("/opt/axon/libaxon_pjrt.so")
        if hook is not None:
            mod.set_axon_ntff_profile_hook(hook)


def run(x, weight, weight_scale, trace=False, trace_cores=None):
    from concourse.bass_utils import run_bass_kernel_spmd

    nc = _get_compiled()

    x = np.ascontiguousarray(np.asarray(x, dtype=np.float32))
    import ml_dtypes

    weight = np.asarray(weight, dtype=np.float32)
    wt = np.ascontiguousarray(weight.T.astype(ml_dtypes.bfloat16))
    weight_scale = np.asarray(weight_scale, dtype=np.float32)
    # [P, KB(bi), OBL(bo)]: s[p, bi, bo] = weight_scale[bo, bi]
    scales_b = np.ascontiguousarray(
        np.broadcast_to(weight_scale.T[None, :, :], (P, KB, OBL)).astype(np.float32)
    )

    in_maps = [
        {
            "x": np.ascontiguousarray(x[c * TSH : (c + 1) * TSH]),
            "wt": wt,
            "s": scales_b,
        }
        for c in range(NCORES)
    ]
    kwargs = {}
    if trace:
        try:
            _ensure_ntff_hook()
        except Exception as e:  # tracing is best-effort; the run still works
            print(f"ntff hook registration failed ({e}); tracing may be skipped")
        kwargs = dict(trace=True, trace_cores=trace_cores or [0])
    res = run_bass_kernel_spmd(nc, in_maps, core_ids=list(range(NCORES)), **kwargs)
    out = np.concatenate([res.results[c]["out"] for c in range(NCORES)], axis=0)
    return out, res


def kernel(x, weight, weight_scale):
    # Rare transient device errors (NRT_EXEC_UNIT_UNRECOVERABLE) have been
    # observed under the profiling path; retry once to be safe.
    try:
        out, _ = run(x, weight, weight_scale)
    except Exception:
        import time

        time.sleep(2)
        out, _ = run(x, weight, weight_scale)
    return out


# revision 28
# speedup vs baseline: 1.0039x; 1.0039x over previous
"""Trainium2 Bass kernel for MockFP8Linear: out = x @ (W * block_scale)^T.

Strategy: data-parallel over tokens across 8 NeuronCores (no collectives).

Layout: the PE contracts along the partition dim, so both operands need
in_features on partitions, but both x and W are stored in_features-
innermost.
  - weight: fed to the device pre-transposed ([in, out] layout — a host-side
    np.ascontiguousarray(weight.T), layout prep only). The dequant scaling
    + bf16 cast happen on-device in one DVE tensor_tensor multiply per
    half-row tile, using a stride-0 broadcast AP for the per-128x128-block
    scales. W^T (bf16, 8 MB) stays resident in SBUF.
  - x: cast f32->bf16 on DVE, then 128x128 blocks are transposed on the
    TensorEngine (transpose-mode matmul against an identity, ~60 ns each
    when batched back-to-back), evicted from PSUM to SBUF by ACT in
    4-block batches. The transposes for token tile tt+1 are woven into
    tile tt's matmul stream so the PE never idles long enough for the HAM
    clock gate to re-throttle.

Main compute runs as two passes over output halves (pass A: o[0:1024]
with the x pipeline woven in, pass B: o[1024:2048] as a pure matmul
stream over the resident x^T tiles — measured at the N=512 issue-rate
floor). lhsT(=x^T block, stationary) @ rhs(=W^T slice, moving, N=512)
bf16 matmuls accumulate fp32 in PSUM over the 16 k-blocks; DVE/ACT evict
to SBUF, DMA out. PSUM accumulator and transpose tiles share one
8-buffer pool.
"""

import os
import sys

import numpy as np

for _p in ("/opt/trn_rl_repo", "/root/.axon_site/_ro/trn_rl_repo"):
    if os.path.isdir(_p) and _p not in sys.path:
        sys.path.append(_p)

TOKENS, IN_F, OUT_F = 16384, 2048, 2048
NCORES = 8
TSH = TOKENS // NCORES  # tokens per core
P = 128
KB = IN_F // P  # contraction blocks
TB = TSH // P  # token tiles per core
OBL = OUT_F // P  # out_features blocks (scale granularity)
NCH = OUT_F // 512  # psum chunks of the output row-tile

_cached = None


def _build():
    from contextlib import ExitStack

    import concourse.tile as tile
    from concourse import bacc, mybir
    from concourse.bass import ds
    from concourse.masks import make_identity

    f32 = mybir.dt.float32
    bf16 = mybir.dt.bfloat16

    nc = bacc.Bacc("TRN2", target_bir_lowering=False, debug=False, num_devices=NCORES)
    x_d = nc.dram_tensor("x", [TSH, IN_F], f32, kind="ExternalInput").ap()
    wt_d = nc.dram_tensor("wt", [IN_F, OUT_F], bf16, kind="ExternalInput").ap()
    s_d = nc.dram_tensor("s", [P, KB, OBL], f32, kind="ExternalInput").ap()
    o_d = nc.dram_tensor("out", [TSH, OUT_F], f32, kind="ExternalOutput").ap()

    with tile.TileContext(nc) as tc:
        with ExitStack() as ctx:
            const = ctx.enter_context(tc.tile_pool(name="const", bufs=1))
            scales = const.tile([P, KB, OBL], f32)
            nc.scalar.dma_start(scales[:], s_d[:])
            ident = const.tile([P, P], bf16)
            make_identity(nc, ident)

            wT_pool = ctx.enter_context(tc.tile_pool(name="wT", bufs=1))
            wTs = [wT_pool.tile([P, OUT_F], bf16, name=f"wT_{ib}") for ib in range(KB)]

            wnat_pool = ctx.enter_context(tc.tile_pool(name="wnat", bufs=3))
            xnat_pool = ctx.enter_context(tc.tile_pool(name="xnat", bufs=3))
            xbf_pool = ctx.enter_context(tc.tile_pool(name="xbf", bufs=3))
            xT_pool = ctx.enter_context(tc.tile_pool(name="xT", bufs=1))
            outsb_pool = ctx.enter_context(tc.tile_pool(name="outsb", bufs=2))
            ops_pool = ctx.enter_context(tc.tile_pool(name="ops", bufs=8, space="PSUM"))
            tps_pool = ops_pool

            def emit_w_half(ib, h):
                wnat = wnat_pool.tile(
                    [P, OUT_F // 2], bf16, tag="wnat", name=f"wnat_{ib}_{h}"
                )
                nc.scalar.dma_start(
                    wnat[:], wt_d[ds(ib * P, P), ds(h * (OUT_F // 2), OUT_F // 2)]
                )
                nc.vector.tensor_tensor(
                    out=wTs[ib][:, ds(h * (OUT_F // 2), OUT_F // 2)].rearrange(
                        "p (b c) -> p b c", c=P
                    ),
                    in0=wnat[:].rearrange("p (b c) -> p b c", c=P),
                    in1=scales[:, ib, ds(h * (OBL // 2), OBL // 2), None].broadcast_to(
                        [P, OBL // 2, P]
                    ),
                    op=mybir.AluOpType.mult,
                )

            # ---- two passes over output halves: pass A computes o[0:1024]
            # for all token tiles with tt+1's load/cast/PE-transposes woven
            # into tt's matmul stream (x^T tiles stay resident); pass B is a
            # pure matmul stream over o[1024:2048]. First MMs need only the
            # first 256 KB of W; W's h=1 halves stream in during pass A. ----
            xbfs = {}
            xTs = [xT_pool.tile([P, IN_F], bf16, name=f"xT_{t}") for t in range(TB)]

            def emit_load_cast(t, chunks=1):
                c = IN_F // chunks
                xnat = xnat_pool.tile([P, IN_F], f32, tag="xnat", name=f"xnat_{t}")
                xbf = xbf_pool.tile([P, IN_F], bf16, tag="xbf", name=f"xbf_{t}")
                for j in range(chunks):
                    nc.sync.dma_start(
                        xnat[:, ds(j * c, c)], x_d[ds(t * P, P), ds(j * c, c)]
                    )
                    nc.vector.tensor_copy(xbf[:, ds(j * c, c)], xnat[:, ds(j * c, c)])
                xbfs[t] = xbf

            def emit_transposes(t, q):
                # quarter q: transpose blocks 4q..4q+3 of token tile t
                tps = tps_pool.tile([P, 4 * P], bf16, tag="ops", name=f"tps_{t}_{q}")
                for j in range(4):
                    ib = 4 * q + j
                    nc.tensor.transpose(
                        tps[:, ds(j * P, P)],
                        xbfs[t][:, ds(ib * P, P)],
                        ident[:],
                    )
                nc.scalar.copy(xTs[t][:, ds(q * 4 * P, 4 * P)], tps[:])

            # prologue: token tile 0 in 512-col chunks so the first
            # transposes start as early as possible
            emit_load_cast(0, chunks=4)
            for q in range(4):
                emit_transposes(0, q)
            emit_load_cast(1)
            for h in range(2):
                for ib in range(KB):
                    emit_w_half(ib, h)

            def half_pass(h, weave):
                for tt in range(TB):
                    xT = xTs[tt]
                    psum = [
                        ops_pool.tile(
                            [P, 512], f32, tag="ops", name=f"ops_{h}_{tt}_{nb}"
                        )
                        for nb in range(2)
                    ]
                    for ib in range(KB):
                        lhsT = xT[:, ds(ib * P, P)]
                        for nb in range(2):
                            nc.tensor.matmul(
                                psum[nb][:],
                                lhsT=lhsT,
                                rhs=wTs[ib][:, ds(h * 1024 + nb * 512, 512)],
                                start=(ib == 0),
                                stop=(ib == KB - 1),
                            )
                        if weave and tt + 1 < TB and ib % 4 == 1:
                            emit_transposes(tt + 1, ib // 4)

                    if weave and tt + 2 < TB:
                        emit_load_cast(tt + 2)
                    outsb = outsb_pool.tile(
                        [P, 1024], f32, tag="outsb", name=f"osb_{h}_{tt}"
                    )
                    # split the eviction across DVE and ACT so the last
                    # tile's drain is half as long
                    nc.vector.tensor_copy(outsb[:, ds(0, 512)], psum[0][:])
                    nc.scalar.copy(outsb[:, ds(512, 512)], psum[1][:])
                    nc.sync.dma_start(
                        o_d[ds(tt * P, P), ds(h * 1024, 1024)], outsb[:]
                    )

            half_pass(0, weave=True)
            half_pass(1, weave=False)

    nc.compile()
    return nc


def _get_compiled():
    global _cached
    if _cached is None:
        _cached = _build()
    return _cached


def _ensure_ntff_hook():
    """Register the axon NTFF profile hook (boot skips it when
    antenv.axon_hooks is absent from the image). Only needed for trace=True."""
    import sys as _sys
    import types as _types

    if "antenv.axon_hooks" not in _sys.modules:
        import antenv

        mod = _types.ModuleType("antenv.axon_hooks")
        mod._hook = None

        def set_axon_ntff_profile_hook(h):
            mod._hook = h

        def get_axon_ntff_profile_hook():
            return mod._hook

        mod.set_axon_ntff_profile_hook = set_axon_ntff_profile_hook
        mod.get_axon_ntff_profile_hook = get_axon_ntff_profile_hook
        _sys.modules["antenv.axon_hooks"] = mod
        antenv.axon_hooks = mod
    mod = _sys.modules["antenv.axon_hooks"]
    if mod._hook is None:
        from trn_agent_boot.trn_boot import _ntff_profile_via_ctypes

        hook = _ntff_profile_via_ctypes("/opt/axon/libaxon_pjrt.so")
        if hook is not None:
            mod.set_axon_ntff_profile_hook(hook)


def run(x, weight, weight_scale, trace=False, trace_cores=None):
    from concourse.bass_utils import run_bass_kernel_spmd

    nc = _get_compiled()

    x = np.ascontiguousarray(np.asarray(x, dtype=np.float32))
    import ml_dtypes

    weight = np.asarray(weight, dtype=np.float32)
    wt = np.ascontiguousarray(weight.T.astype(ml_dtypes.bfloat16))
    weight_scale = np.asarray(weight_scale, dtype=np.float32)
    # [P, KB(bi), OBL(bo)]: s[p, bi, bo] = weight_scale[bo, bi]
    scales_b = np.ascontiguousarray(
        np.broadcast_to(weight_scale.T[None, :, :], (P, KB, OBL)).astype(np.float32)
    )

    in_maps = [
        {
            "x": np.ascontiguousarray(x[c * TSH : (c + 1) * TSH]),
            "wt": wt,
            "s": scales_b,
        }
        for c in range(NCORES)
    ]
    kwargs = {}
    if trace:
        try:
            _ensure_ntff_hook()
        except Exception as e:  # tracing is best-effort; the run still works
            print(f"ntff hook registration failed ({e}); tracing may be skipped")
        kwargs = dict(trace=True, trace_cores=trace_cores or [0])
    res = run_bass_kernel_spmd(nc, in_maps, core_ids=list(range(NCORES)), **kwargs)
    out = np.concatenate([res.results[c]["out"] for c in range(NCORES)], axis=0)
    return out, res


def kernel(x, weight, weight_scale):
    # Rare transient device errors (NRT_EXEC_UNIT_UNRECOVERABLE) have been
    # observed under the profiling path; retry once to be safe.
    try:
        out, _ = run(x, weight, weight_scale)
    except Exception:
        import time

        time.sleep(2)
        out, _ = run(x, weight, weight_scale)
    return out
